# revision 8
# baseline (speedup 1.0000x reference)
"""Bass/Trainium2 kernel for nn_CLUBForCategorical (8-core SPMD).

Math: with lp = log_softmax(x @ W.T + b, axis=-1),
    positive = mean_i lp[i, labels[i]]
    negative = mean_j (mean_i lp)[labels[j]]
    out      = positive - negative

The lse_i terms (and bias b) cancel exactly in positive - negative:

    out = (1/N) * sum_i x_i . W[labels_i]  -  (1/N^2) * xsum . Sg
    xsum = sum_i x_i,   Sg = sum_j W[labels_j]

so no logits/softmax is needed (verified to 2e-13 vs the f64 softmax
reference, including nonzero b).

Sharding: rows are routed to cores by CLASS (snake-deal classes by
count into 8 bins, flatten, exact-cut at 2048-row boundaries splitting
boundary classes), so every core gets exactly 2048 rows (16 tiles of
128) and <=128 distinct classes; per-core partials are additive even
for split classes. x ships as bf16 (tolerance is 2e-2; bf16-only x
gives 5.7e-3 on this data, checked against f64 on host).

Per core: G = onehot^T @ x accumulated on the PE (eq tiles from an
iota/is_equal compare on DVE, two 512-col PSUM banks), then
diag_c = sum(G .* Wslice) (DVE + Pool dots), U_c = per-128-block
column sums of G (tiny matmuls via an SBUF copy of G), and
V_c = hist^T @ Wslice (tiny matmuls, mid-stream). Each core writes one
[128, 18] payload (V | U | dcol pieces); the host sums the 8 payloads
and takes the final 1024-dot -- the gather/unshard step (the per-core
DMA issue order is ring-aware: HWDGE rings recycle with ~2.2us re-arm
latency, so x goes out as interleaved jumbo loads on both HWDGE queues
with wslice on the SWDGE queue).

Fallback for pathological label distributions (>128 classes in a
chunk): the previous bucket-sharded kernel (label//125 routing, hi/lo
bf16 split of x, device AllGather), which handles any distribution.
"""

import sys

import numpy as np

if "/opt/trn_rl_repo" not in sys.path:
    sys.path.insert(0, "/opt/trn_rl_repo")

N, D, L = 16384, 1024, 1000
CORES = 8
NLOC = N // CORES          # 2048 rows per core

_CACHE: dict = {}

_RUN_KW: dict = {}   # test harness may set e.g. {"trace": True}
_FAST_KW: dict = {}  # overrides for _build_nc_fast (dev/bisect)
LAST_RESULT = None   # BassKernelResults of the most recent run


# ---------------------------------------------------------------------------
# fast path: class-balanced sharding, bf16 x, no collective
# ---------------------------------------------------------------------------

def _issue_fast(nt, split_head=False):
    """DMA issue order: lh first, x0/x1 single (fast PE start; optionally
    as half-column loads), rest as 2-tile jumbos alternating HWDGE
    queues, wslice on SWDGE."""
    order = [("sync", "lh")]
    if split_head:
        order += [("sync", ("xh", 0, 0)), ("scalar", ("xh", 1, 0)),
                  ("sync", ("xh", 0, 1)), ("scalar", ("xh", 1, 1))]
    else:
        order += [("sync", ("x", 0, 1)), ("scalar", ("x", 1, 1))]
    order += [("gpsimd", "w")]
    q = ["sync", "scalar"]
    k = 0
    t = 2
    while t < nt:
        cnt = min(2, nt - t)
        order.append((q[k % 2], ("x", t, cnt)))
        k += 1
        t += cnt
    return order


def _build_nc_fast(nt=16, junk=(0, 2), issue=None, groups=None):
    import concourse.bacc as bacc
    import concourse.mybir as mybir
    import concourse.tile as tile

    f32 = mybir.dt.float32
    bf16 = mybir.dt.bfloat16
    mult = mybir.AluOpType.mult
    bypass = mybir.AluOpType.bypass
    is_eq = mybir.AluOpType.is_equal
    Copy = mybir.ActivationFunctionType.Copy

    if groups is None:
        groups = (nt,)
    ng = len(groups)
    p_cols = 8 + 8 * ng + 2 * ng

    nc = bacc.Bacc("TRN2", target_bir_lowering=False, debug=False,
                   num_devices=CORES)

    x_d = nc.dram_tensor("x", [128, nt * D], bf16, kind="ExternalInput")
    w_d = nc.dram_tensor("wslice", [128, D], f32, kind="ExternalInput")
    lh_d = nc.dram_tensor("lh", [128, nt + 1], f32, kind="ExternalInput")
    outp_d = nc.dram_tensor("out_p", [128, p_cols], f32,
                            kind="ExternalOutput")

    if issue is None:
        issue = _issue_fast(nt)

    with tile.TileContext(nc) as tc:
        with (
            tc.tile_pool(name="xp", bufs=nt) as xp,
            tc.tile_pool(name="eqp", bufs=nt) as eqp,
            tc.tile_pool(name="small", bufs=1) as small,
            tc.tile_pool(name="ps", bufs=1, space="PSUM") as ps,
        ):
            engs = {"sync": nc.sync, "scalar": nc.scalar, "gpsimd": nc.gpsimd}

            junk_sb = small.tile([128, 512], bf16, tag="junk")
            nc.vector.memset(junk_sb[:], 0.0)
            ones = small.tile([128, 1], f32, tag="ones")
            nc.vector.memset(ones[:], 1.0)
            # iota first on the Pool queue (before any gpsimd DMA gen)
            iota_i = small.tile([128, 128], mybir.dt.int32, tag="iota_i")
            nc.gpsimd.iota(iota_i[:], pattern=[[1, 128]], channel_multiplier=0)
            iota_f = small.tile([128, 128], f32, tag="iota_f")
            nc.vector.tensor_copy(iota_f[:], iota_i[:])

            lh = small.tile([128, nt + 1], f32, tag="lh")
            wsl = small.tile([128, D], f32, tag="wsl")

            xap = [None] * nt   # tile -> (ap_cols_0_512, ap_cols_512_1024)
            for ename, op in issue:
                eng = engs[ename]
                if op == "lh":
                    eng.dma_start(out=lh[:], in_=lh_d[:])
                elif op == "w":
                    eng.dma_start(out=wsl[:], in_=w_d[:])
                elif op[0] == "xh":
                    _, t0, h = op
                    buf = xp.tile([128, 512], bf16, tag="x",
                                  name=f"x{t0}h{h}")
                    eng.dma_start(
                        out=buf[:],
                        in_=x_d[:, t0 * D + 512 * h : t0 * D + 512 * (h + 1)])
                    cur = list(xap[t0]) if xap[t0] else [None, None]
                    cur[h] = buf[:]
                    xap[t0] = tuple(cur)
                else:
                    _, t0, cnt = op
                    buf = xp.tile([128, cnt * D], bf16, tag="x",
                                  name=f"x{t0}_{cnt}")
                    eng.dma_start(out=buf[:],
                                  in_=x_d[:, t0 * D : (t0 + cnt) * D])
                    for k in range(cnt):
                        xap[t0 + k] = (buf[:, k * D : k * D + 512],
                                       buf[:, k * D + 512 : (k + 1) * D])
            assert all(a is not None and all(x is not None for x in a)
                       for a in xap)

            eqs = []
            for t in range(nt):
                eq = eqp.tile([128, 128], bf16, tag="eq", name=f"eq{t}")
                nc.vector.tensor_scalar(
                    out=eq[:], in0=iota_f[:], scalar1=lh[:, t : t + 1],
                    scalar2=None, op0=is_eq,
                )
                eqs.append(eq)

            pg = [(ps.tile([128, 512], f32, tag=f"p_{g}0", name=f"p_{g}0"),
                   ps.tile([128, 512], f32, tag=f"p_{g}1", name=f"p_{g}1"))
                  for g in range(ng)]
            pay_ps = ps.tile([128, 8 + 8 * ng], f32, tag="pay_ps")
            p_warm = ps.tile([1, 512], f32, tag="p_warm")

            n64, n512 = junk
            for _ in range(n64):
                nc.tensor.matmul(p_warm[0:1, 0:64], junk_sb[:, 0:1],
                                 junk_sb[:, 0:64], start=True, stop=True)
            for _ in range(n512):
                nc.tensor.matmul(p_warm[:], junk_sb[:, 0:1], junk_sb[:],
                                 start=True, stop=True)

            pay_sb = small.tile([128, p_cols], f32, tag="pay_sb")
            nc.vector.memset(pay_sb[:], 0.0)
            g_sb = small.tile([128, D], f32, tag="g_sb")
            scr = small.tile([128, D], f32, tag="scr")

            hist = lh[:, nt : nt + 1]
            v_done = False

            def emit_v():
                for j in range(8):
                    blk = slice(128 * j, 128 * (j + 1))
                    nc.tensor.matmul(pay_ps[:, j : j + 1], wsl[:, blk],
                                     hist, start=True, stop=True)
                nc.vector.tensor_copy(pay_sb[:, 0:8], pay_ps[:, 0:8])

            gstart = 0
            for g, gend in enumerate(groups):
                p0, p1 = pg[g]
                tiles = list(range(gstart, gend))
                for i, t in enumerate(tiles):
                    first, last = (i == 0), (i == len(tiles) - 1)
                    nc.tensor.matmul(p0[:], eqs[t][:], xap[t][0],
                                     start=first, stop=last)
                    nc.tensor.matmul(p1[:], eqs[t][:], xap[t][1],
                                     start=first, stop=last)
                    if g == 0 and i == 3 and not v_done:
                        emit_v()
                        v_done = True
                gstart = gend

                dbase = 8 + 8 * ng + 2 * g
                # group tail: both PSUM->SBUF copies on Act (only Act/DVE
                # may read PSUM; keeping DVE free lets the diag dots start
                # right as each g_sb half lands), per-half dots on DVE, U
                # matmuls from g_sb on PE
                for h, ph in ((0, p0), (1, p1)):
                    cs = slice(512 * h, 512 * (h + 1))
                    nc.scalar.activation(g_sb[:, cs], ph[:], Copy)
                    nc.vector.scalar_tensor_tensor(
                        out=scr[:, cs], in0=g_sb[:, cs], scalar=1.0,
                        in1=wsl[:, cs], op0=bypass, op1=mult,
                        accum_out=pay_sb[:, dbase + h : dbase + h + 1],
                    )
                    for j in range(4 * h, 4 * h + 4):
                        blk = slice(128 * j, 128 * (j + 1))
                        nc.tensor.matmul(
                            pay_ps[:, 8 + 8 * g + j : 9 + 8 * g + j],
                            g_sb[:, blk], ones[:], start=True, stop=True)
                # Act does the U paycopy (GPSIMD cannot access PSUM; DVE
                # is busy with the diag dot here)
                nc.scalar.activation(
                    pay_sb[:, 8 + 8 * g : 16 + 8 * g],
                    pay_ps[:, 8 + 8 * g : 16 + 8 * g], Copy)
                if not v_done:
                    emit_v()
                    v_done = True

            nc.sync.dma_start(out=outp_d[:], in_=pay_sb[:])

    nc.compile()
    return nc


def _prep_fast(x, lab, w):
    """Class-balanced sharding. Returns (in_maps, nt) or (None, 0) if a
    chunk would need more than 128 distinct classes."""
    import ml_dtypes

    counts = np.bincount(lab, minlength=L)
    order = np.argsort(-counts, kind="stable")
    bins = [[] for _ in range(CORES)]
    for i, cl in enumerate(order):
        r = i % (2 * CORES)
        b = r if r < CORES else 2 * CORES - 1 - r
        bins[b].append(cl)
    seq = [cl for b in bins for cl in b]

    core_classes = [[] for _ in range(CORES)]
    c, filled = 0, 0
    for cl in seq:
        n = int(counts[cl])
        while n > 0:
            take = min(n, NLOC - filled)
            core_classes[c].append((int(cl), take))
            filled += take
            n -= take
            if filled == NLOC:
                c += 1
                filled = 0
    if max(len(cc) for cc in core_classes) > 128:
        return None, 0
    nt = NLOC // 128  # 16

    row_order = np.argsort(lab, kind="stable")
    class_start = np.zeros(L + 1, np.int64)
    class_start[1:] = np.cumsum(counts)
    taken = np.zeros(L, np.int64)

    maps = []
    for cc in core_classes:
        rows = np.empty(NLOC, np.int64)
        lrel = np.empty(NLOC, np.float32)
        wsl = np.zeros((128, D), np.float32)
        hist = np.zeros((128, 1), np.float32)
        pos = 0
        for slot, (cl, take) in enumerate(cc):
            s = class_start[cl] + taken[cl]
            rows[pos : pos + take] = row_order[s : s + take]
            lrel[pos : pos + take] = slot
            taken[cl] += take
            wsl[slot] = w[cl]
            hist[slot, 0] = take
            pos += take
        assert pos == NLOC
        xb = x[rows].astype(ml_dtypes.bfloat16)
        x_t = np.ascontiguousarray(
            xb.reshape(nt, 128, D).transpose(1, 0, 2).reshape(128, nt * D)
        )
        lh = np.empty((128, nt + 1), np.float32)
        lh[:, 0:nt] = lrel.reshape(nt, 128).T
        lh[:, nt] = hist[:, 0]
        maps.append({"x": x_t, "wslice": wsl, "lh": lh})
    return maps, nt


def _combine_fast(results, ng=1):
    """Sum per-core payloads (the unshard step) and finish the scalar."""
    U = np.zeros(D, np.float64)
    V = np.zeros(D, np.float64)
    diag = 0.0
    for r in results:
        pay = np.asarray(r["out_p"], np.float64)
        V += pay[:, 0:8].T.reshape(D)
        for g in range(ng):
            U += pay[:, 8 + 8 * g : 16 + 8 * g].T.reshape(D)
        db = 8 + 8 * ng
        diag += pay[:, db : db + 2 * ng].sum()
    out = diag / N - (U @ V) / (float(N) * float(N))
    return np.float32(out)


def kernel(inputs, labels, W, b):
    global LAST_RESULT
    import os

    # The run path needs the axon trn2 PJRT backend; drop a cpu pin if jax
    # hasn't been initialized yet (the reference is jax-on-cpu friendly).
    if "jax" not in sys.modules and os.environ.get("JAX_PLATFORMS") == "cpu":
        del os.environ["JAX_PLATFORMS"]

    from concourse.bass_utils import run_bass_kernel_spmd

    x = np.ascontiguousarray(np.asarray(inputs, dtype=np.float32))
    lab = np.asarray(labels).astype(np.int64)
    w = np.ascontiguousarray(np.asarray(W, dtype=np.float32))
    assert x.shape == (N, D) and w.shape == (L, D) and lab.shape == (N,)
    assert lab.min() >= 0 and lab.max() < L

    in_maps, nt = _prep_fast(x, lab, w)
    if in_maps is not None:
        key = f"nc_fast_{nt}_{sorted(_FAST_KW.items())}"
        if key not in _CACHE:
            _CACHE[key] = _build_nc_fast(nt=nt, **_FAST_KW)
        nc = _CACHE[key]
        res = run_bass_kernel_spmd(nc, in_maps, list(range(CORES)), **_RUN_KW)
        LAST_RESULT = res
        out = _combine_fast([res.results[c] for c in range(CORES)])
        return np.asarray(out, dtype=np.float32)

    # fallback: bucket-sharded kernel (any label distribution)
    in_maps, btiles = _prep_bucket(x, lab, w)
    assert in_maps is not None, "bucket prep failed"
    key = f"nc_bucket_{btiles}"
    if key not in _CACHE:
        _CACHE[key] = _build_nc_bucket(btiles)
    nc = _CACHE[key]
    res = run_bass_kernel_spmd(nc, in_maps, list(range(CORES)), **_RUN_KW)
    LAST_RESULT = res
    out = np.float32(res.results[0]["out"][0, 0])
    return np.asarray(out, dtype=np.float32)


if __name__ == "__main__":
    import reference

    inp = reference.setup_inputs()
    expected = np.asarray(reference.reference(**inp))
    actual = kernel(**{k: np.asarray(v) for k, v in inp.items()})
    rel = abs(float(actual) - float(expected)) / max(abs(float(expected)), 1e-30)
    print("expected:", expected, "actual:", actual, "rel err:", rel)


# ---------------------------------------------------------------------------
# Bucket-sharded fallback (from the previous kernel): host routes rows to
# cores by class range (125 classes/core). Handles any distribution with
# btiles sized to the fullest bucket.
# ---------------------------------------------------------------------------

LPC = L // CORES            # 125 classes per core
PAY = 17                    # payload cols: U[0:8] | V[8:16] | diag[16]


def _build_nc_bucket(btiles, big_bufs=4):
    import concourse.bacc as bacc
    import concourse.mybir as mybir
    import concourse.tile as tile

    f32 = mybir.dt.float32
    bf16 = mybir.dt.bfloat16
    mult = mybir.AluOpType.mult
    bypass = mybir.AluOpType.bypass
    is_eq = mybir.AluOpType.is_equal

    nc = bacc.Bacc(
        "TRN2",
        target_bir_lowering=False,
        debug=False,
        num_devices=CORES,
    )
    bf16_ = mybir.dt.bfloat16
    x2_d = nc.dram_tensor("x2", [128, btiles * 2 * D], bf16_,
                          kind="ExternalInput")
    w_d = nc.dram_tensor("wslice", [128, D], f32, kind="ExternalInput")
    lrel_d = nc.dram_tensor("lrel", [128, btiles], f32, kind="ExternalInput")
    hist_d = nc.dram_tensor("hist", [128, 1], f32, kind="ExternalInput")
    out_d = nc.dram_tensor("out", [1, 1], f32, kind="ExternalOutput")

    x2_ch = x2_d[:].rearrange("p (c d) -> c p d", c=btiles)

    with tile.TileContext(nc) as tc:
        with (
            tc.tile_pool(name="big", bufs=big_bufs) as big,
            tc.tile_pool(name="small", bufs=1) as small,
            tc.tile_pool(name="eqp", bufs=3) as eqp,
            tc.tile_pool(name="ps", bufs=1, space="PSUM") as ps,
            tc.tile_pool(name="dram", bufs=1, space="DRAM") as dram,
        ):
            slots = small.tile([128, 8 * PAY], f32, tag="slots")
            pay_sb = small.tile([128, PAY], f32, tag="pay_sb")
            nc.vector.memset(pay_sb[:], 0.0)
            ones = small.tile([128, 1], f32, tag="ones")
            nc.vector.memset(ones[:], 1.0)
            junk = small.tile([128, 512], bf16, tag="junk")
            nc.vector.memset(junk[:], 0.0)
            lrel = small.tile([128, btiles], f32, tag="lrel")
            nc.scalar.dma_start(out=lrel[:], in_=lrel_d[:])
            hist = small.tile([128, 1], f32, tag="hist")
            nc.scalar.dma_start(out=hist[:], in_=hist_d[:])
            wsl = small.tile([128, D], f32, tag="wsl")
            nc.gpsimd.dma_start(out=wsl[:], in_=w_d[:])
            iota_i = small.tile([128, 128], mybir.dt.int32, tag="iota_i")
            nc.gpsimd.iota(iota_i[:], pattern=[[1, 128]], channel_multiplier=0)
            iota_f = small.tile([128, 128], f32, tag="iota_f")
            nc.vector.tensor_copy(iota_f[:], iota_i[:])

            p_g0 = ps.tile([128, 512], f32, tag="p_g0")
            p_g1 = ps.tile([128, 512], f32, tag="p_g1")
            pay_ps = ps.tile([128, PAY], f32, tag="pay_ps")
            p_fin = ps.tile([1, 1], f32, tag="p_fin")
            p_warm = ps.tile([1, 512], f32, tag="p_warm")

            for _ in range(7):
                nc.tensor.matmul(p_warm[:], junk[:, 0:1], junk[:],
                                 start=True, stop=True)

            for t in range(btiles):
                first, last = (t == 0), (t == btiles - 1)
                x2 = big.tile([128, 2 * D], bf16, tag="x2", name=f"x2{t}")
                eng = nc.sync if t % 2 == 0 else nc.scalar
                eng.dma_start(out=x2[:], in_=x2_ch[t])
                xh = x2[:, 0:D]
                xl = x2[:, D : 2 * D]
                eq = eqp.tile([128, 128], bf16, tag="eq", name=f"eq{t}")
                nc.vector.tensor_scalar(
                    out=eq[:], in0=iota_f[:], scalar1=lrel[:, t : t + 1],
                    scalar2=None, op0=is_eq,
                )
                nc.tensor.matmul(p_g0[:], eq[:], xh[:, 0:512],
                                 start=first, stop=False)
                nc.tensor.matmul(p_g0[:], eq[:], xl[:, 0:512],
                                 start=False, stop=last)
                nc.tensor.matmul(p_g1[:], eq[:], xh[:, 512:1024],
                                 start=first, stop=False)
                nc.tensor.matmul(p_g1[:], eq[:], xl[:, 512:1024],
                                 start=False, stop=last)

            g_sb = small.tile([128, D], f32, tag="g_sb")
            nc.vector.tensor_copy(g_sb[:, 0:512], p_g0[:])
            nc.vector.tensor_copy(g_sb[:, 512:1024], p_g1[:])

            prod = small.tile([128, D], f32, tag="prod")
            diag_col = small.tile([128, 1], f32, tag="diag_col")
            nc.vector.scalar_tensor_tensor(
                out=prod[:], in0=g_sb[:], scalar=1.0, in1=wsl[:],
                op0=bypass, op1=mult, accum_out=diag_col[:],
            )

            for j in range(8):
                blk = slice(128 * j, 128 * (j + 1))
                nc.tensor.matmul(pay_ps[:, j : j + 1], g_sb[:, blk], ones[:],
                                 start=True, stop=True)
                nc.tensor.matmul(pay_ps[:, 8 + j : 9 + j], wsl[:, blk],
                                 hist[:], start=True, stop=True)
            nc.tensor.matmul(pay_ps[0:1, 16:17], diag_col[:], ones[:],
                             start=True, stop=True)

            nc.vector.tensor_copy(pay_sb[:, 0:16], pay_ps[:, 0:16])
            nc.vector.tensor_copy(pay_sb[0:1, 16:17], pay_ps[0:1, 16:17])
            cc_in = dram.tile([128, PAY], f32, tag="cc_in")
            cc_out = dram.tile([128 * CORES, PAY], f32, tag="cc_out")
            nc.scalar.dma_start(out=cc_in[:], in_=pay_sb[:])
            nc.gpsimd.collective_compute(
                "AllGather",
                bypass,
                replica_groups=[list(range(CORES))],
                ins=[cc_in[:].opt()],
                outs=[cc_out[:].opt()],
            )
            gath = cc_out[:].rearrange("(g p) j -> p g j", g=CORES)
            nc.scalar.dma_start(
                out=slots[:].rearrange("p (g j) -> p g j", g=CORES), in_=gath
            )

            nc.vector.tensor_add(slots[:, 0 : 4 * PAY], slots[:, 0 : 4 * PAY],
                                 slots[:, 4 * PAY : 8 * PAY])
            nc.vector.tensor_add(slots[:, 0 : 2 * PAY], slots[:, 0 : 2 * PAY],
                                 slots[:, 2 * PAY : 4 * PAY])
            nc.vector.tensor_add(slots[:, 0:PAY], slots[:, 0:PAY],
                                 slots[:, PAY : 2 * PAY])

            prod8 = small.tile([128, 8], f32, tag="prod8")
            dot_col = small.tile([128, 1], f32, tag="dot_col")
            nc.vector.scalar_tensor_tensor(
                out=prod8[:], in0=slots[:, 0:8], scalar=1.0,
                in1=slots[:, 8:16], op0=bypass, op1=mult,
                accum_out=dot_col[:],
            )
            nc.tensor.matmul(p_fin[:], dot_col[:], ones[:],
                             start=True, stop=True)
            tmp = small.tile([1, 1], f32, tag="tmp")
            ans = small.tile([1, 1], f32, tag="ans")
            inv_n = 1.0 / float(N)
            nc.vector.tensor_scalar_mul(tmp[:], p_fin[:], inv_n)
            nc.vector.tensor_sub(ans[:], slots[0:1, 16:17], tmp[:])
            nc.vector.tensor_scalar_mul(ans[:], ans[:], inv_n)
            nc.sync.dma_start(out=out_d[:], in_=ans[:])

    nc.compile()
    return nc


def _prep_bucket(x, lab, w):
    """Route rows to cores by label // LPC; btiles sized to the fullest
    bucket (with hi/lo bf16 split so this fallback is near-exact)."""
    core_of = lab // LPC
    counts = np.bincount(core_of, minlength=CORES)
    btiles = max(16, -(-int(counts.max()) // 128))
    maps = []
    for c in range(CORES):
        rows = np.nonzero(core_of == c)[0]
        nb = len(rows)
        import ml_dtypes
        xb = np.zeros((btiles * 128, D), np.float32)
        xb[:nb] = x[rows]
        xh = xb.astype(ml_dtypes.bfloat16)
        xl = (xb - xh.astype(np.float32)).astype(ml_dtypes.bfloat16)
        lrel = np.full((btiles * 128,), -1.0, np.float32)
        lrel[:nb] = (lab[rows] - c * LPC).astype(np.float32)
        wsl = np.zeros((128, D), np.float32)
        wsl[:LPC] = w[c * LPC : (c + 1) * LPC]
        x2 = np.concatenate(
            [xh.reshape(btiles, 128, D), xl.reshape(btiles, 128, D)], axis=2
        ).transpose(1, 0, 2).reshape(128, btiles * 2 * D)

        hist = np.zeros((128, 1), np.float32)
        cnt = np.bincount(lab[rows] - c * LPC, minlength=LPC)
        hist[:LPC, 0] = cnt.astype(np.float32)

        maps.append({
            "x2": np.ascontiguousarray(x2),
            "lrel": np.ascontiguousarray(
                lrel.reshape(btiles, 128).T),
            "wslice": wsl,
            "hist": hist,
        })
    return maps, btiles


# revision 11
# speedup vs baseline: 1.0004x; 1.0004x over previous
"""Bass/Trainium2 kernel for nn_CLUBForCategorical (8-core SPMD).

Math: with lp = log_softmax(x @ W.T + b, axis=-1),
    positive = mean_i lp[i, labels[i]]
    negative = mean_j (mean_i lp)[labels[j]]
    out      = positive - negative

The lse_i terms (and bias b) cancel exactly in positive - negative:

    out = (1/N) * sum_i x_i . W[labels_i]  -  (1/N^2) * xsum . Sg
    xsum = sum_i x_i,   Sg = sum_j W[labels_j]

so no logits/softmax is needed (verified to 2e-13 vs the f64 softmax
reference, including nonzero b).

Sharding: rows are routed to cores by CLASS (snake-deal classes by
count into 8 bins, flatten, exact-cut at 2048-row boundaries splitting
boundary classes), so every core gets exactly 2048 rows (16 tiles of
128) and <=128 distinct classes; per-core partials are additive even
for split classes. x ships as bf16 (tolerance is 2e-2; bf16-only x
gives 5.7e-3 on this data, checked against f64 on host).

Per core: G = onehot^T @ x accumulated on the PE (eq tiles from an
iota/is_equal compare on DVE, two 512-col PSUM banks), then
diag_c = sum(G .* Wslice) (DVE + Pool dots), U_c = per-128-block
column sums of G (tiny matmuls via an SBUF copy of G), and
V_c = hist^T @ Wslice (tiny matmuls, mid-stream). Each core writes one
[128, 18] payload (V | U | dcol pieces); the host sums the 8 payloads
and takes the final 1024-dot -- the gather/unshard step (the per-core
DMA issue order is ring-aware: HWDGE rings recycle with ~2.2us re-arm
latency, so x goes out as interleaved jumbo loads on both HWDGE queues
with wslice on the SWDGE queue).

Fallback for pathological label distributions (>128 classes in a
chunk): the previous bucket-sharded kernel (label//125 routing, hi/lo
bf16 split of x, device AllGather), which handles any distribution.
"""

import sys

import numpy as np

if "/opt/trn_rl_repo" not in sys.path:
    sys.path.insert(0, "/opt/trn_rl_repo")

N, D, L = 16384, 1024, 1000
CORES = 8
NLOC = N // CORES          # 2048 rows per core

_CACHE: dict = {}

_RUN_KW: dict = {}   # test harness may set e.g. {"trace": True}
_FAST_KW: dict = {}  # overrides for _build_nc_fast (dev/bisect)
LAST_RESULT = None   # BassKernelResults of the most recent run


# ---------------------------------------------------------------------------
# fast path: class-balanced sharding, bf16 x, no collective
# ---------------------------------------------------------------------------

def _issue_fast(nt, split_head=False):
    """DMA issue order: lh first, x0/x1 single (fast PE start; optionally
    as half-column loads), rest as 2-tile jumbos alternating HWDGE
    queues, wslice on SWDGE."""
    order = [("sync", "lh")]
    if split_head:
        order += [("sync", ("xh", 0, 0)), ("scalar", ("xh", 1, 0)),
                  ("sync", ("xh", 0, 1)), ("scalar", ("xh", 1, 1))]
    else:
        order += [("sync", ("x", 0, 1)), ("scalar", ("x", 1, 1))]
    order += [("gpsimd", "w")]
    q = ["sync", "scalar"]
    k = 0
    t = 2
    while t < nt:
        cnt = min(2, nt - t)
        order.append((q[k % 2], ("x", t, cnt)))
        k += 1
        t += cnt
    return order


def _build_nc_fast(nt=16, junk=(16, 0), issue=None, groups=None):
    import concourse.bacc as bacc
    import concourse.mybir as mybir
    import concourse.tile as tile

    f32 = mybir.dt.float32
    bf16 = mybir.dt.bfloat16
    mult = mybir.AluOpType.mult
    bypass = mybir.AluOpType.bypass
    is_eq = mybir.AluOpType.is_equal
    Copy = mybir.ActivationFunctionType.Copy

    if groups is None:
        groups = (nt,)
    ng = len(groups)
    p_cols = 8 + 8 * ng + 2 * ng

    nc = bacc.Bacc("TRN2", target_bir_lowering=False, debug=False,
                   num_devices=CORES)

    x_d = nc.dram_tensor("x", [128, nt * D], bf16, kind="ExternalInput")
    w_d = nc.dram_tensor("wslice", [128, D], f32, kind="ExternalInput")
    lh_d = nc.dram_tensor("lh", [128, nt + 1], f32, kind="ExternalInput")
    outp_d = nc.dram_tensor("out_p", [128, p_cols], f32,
                            kind="ExternalOutput")

    if issue is None:
        issue = _issue_fast(nt)

    with tile.TileContext(nc) as tc:
        with (
            tc.tile_pool(name="xp", bufs=nt) as xp,
            tc.tile_pool(name="eqp", bufs=nt) as eqp,
            tc.tile_pool(name="small", bufs=1) as small,
            tc.tile_pool(name="ps", bufs=1, space="PSUM") as ps,
        ):
            engs = {"sync": nc.sync, "scalar": nc.scalar, "gpsimd": nc.gpsimd}

            junk_sb = small.tile([128, 512], bf16, tag="junk")
            nc.vector.memset(junk_sb[:], 0.0)
            ones = small.tile([128, 1], f32, tag="ones")
            nc.vector.memset(ones[:], 1.0)
            # iota first on the Pool queue (before any gpsimd DMA gen)
            iota_i = small.tile([128, 128], mybir.dt.int32, tag="iota_i")
            nc.gpsimd.iota(iota_i[:], pattern=[[1, 128]], channel_multiplier=0)
            iota_f = small.tile([128, 128], f32, tag="iota_f")
            nc.vector.tensor_copy(iota_f[:], iota_i[:])

            lh = small.tile([128, nt + 1], f32, tag="lh")
            wsl = small.tile([128, D], f32, tag="wsl")

            xap = [None] * nt   # tile -> (ap_cols_0_512, ap_cols_512_1024)
            for ename, op in issue:
                eng = engs[ename]
                if op == "lh":
                    eng.dma_start(out=lh[:], in_=lh_d[:])
                elif op == "w":
                    eng.dma_start(out=wsl[:], in_=w_d[:])
                elif op[0] == "xh":
                    _, t0, h = op
                    buf = xp.tile([128, 512], bf16, tag="x",
                                  name=f"x{t0}h{h}")
                    eng.dma_start(
                        out=buf[:],
                        in_=x_d[:, t0 * D + 512 * h : t0 * D + 512 * (h + 1)])
                    cur = list(xap[t0]) if xap[t0] else [None, None]
                    cur[h] = buf[:]
                    xap[t0] = tuple(cur)
                else:
                    _, t0, cnt = op
                    buf = xp.tile([128, cnt * D], bf16, tag="x",
                                  name=f"x{t0}_{cnt}")
                    eng.dma_start(out=buf[:],
                                  in_=x_d[:, t0 * D : (t0 + cnt) * D])
                    for k in range(cnt):
                        xap[t0 + k] = (buf[:, k * D : k * D + 512],
                                       buf[:, k * D + 512 : (k + 1) * D])
            assert all(a is not None and all(x is not None for x in a)
                       for a in xap)

            eqs = []
            for t in range(nt):
                eq = eqp.tile([128, 128], bf16, tag="eq", name=f"eq{t}")
                nc.vector.tensor_scalar(
                    out=eq[:], in0=iota_f[:], scalar1=lh[:, t : t + 1],
                    scalar2=None, op0=is_eq,
                )
                eqs.append(eq)

            pg = [(ps.tile([128, 512], f32, tag=f"p_{g}0", name=f"p_{g}0"),
                   ps.tile([128, 512], f32, tag=f"p_{g}1", name=f"p_{g}1"))
                  for g in range(ng)]
            pay_ps = ps.tile([128, 8 + 8 * ng], f32, tag="pay_ps")
            p_warm = ps.tile([1, 512], f32, tag="p_warm")

            n64, n512 = junk
            for _ in range(n64):
                nc.tensor.matmul(p_warm[0:1, 0:64], junk_sb[:, 0:1],
                                 junk_sb[:, 0:64], start=True, stop=True)
            for _ in range(n512):
                nc.tensor.matmul(p_warm[:], junk_sb[:, 0:1], junk_sb[:],
                                 start=True, stop=True)

            pay_sb = small.tile([128, p_cols], f32, tag="pay_sb")
            nc.vector.memset(pay_sb[:], 0.0)
            g_sb = small.tile([128, D], f32, tag="g_sb")
            scr = small.tile([128, D], f32, tag="scr")

            hist = lh[:, nt : nt + 1]
            v_done = False

            def emit_v():
                for j in range(8):
                    blk = slice(128 * j, 128 * (j + 1))
                    nc.tensor.matmul(pay_ps[:, j : j + 1], wsl[:, blk],
                                     hist, start=True, stop=True)
                nc.vector.tensor_copy(pay_sb[:, 0:8], pay_ps[:, 0:8])

            gstart = 0
            for g, gend in enumerate(groups):
                p0, p1 = pg[g]
                tiles = list(range(gstart, gend))
                for i, t in enumerate(tiles):
                    first, last = (i == 0), (i == len(tiles) - 1)
                    nc.tensor.matmul(p0[:], eqs[t][:], xap[t][0],
                                     start=first, stop=last)
                    nc.tensor.matmul(p1[:], eqs[t][:], xap[t][1],
                                     start=first, stop=last)
                    if g == 0 and i == 3 and not v_done:
                        emit_v()
                        v_done = True
                gstart = gend

                dbase = 8 + 8 * ng + 2 * g
                # group tail: both PSUM->SBUF copies on Act (only Act/DVE
                # may read PSUM; keeping DVE free lets the diag dots start
                # right as each g_sb half lands), per-half dots on DVE, U
                # matmuls from g_sb on PE
                for h, ph in ((0, p0), (1, p1)):
                    cs = slice(512 * h, 512 * (h + 1))
                    nc.scalar.activation(g_sb[:, cs], ph[:], Copy)
                    nc.vector.scalar_tensor_tensor(
                        out=scr[:, cs], in0=g_sb[:, cs], scalar=1.0,
                        in1=wsl[:, cs], op0=bypass, op1=mult,
                        accum_out=pay_sb[:, dbase + h : dbase + h + 1],
                    )
                    for j in range(4 * h, 4 * h + 4):
                        blk = slice(128 * j, 128 * (j + 1))
                        nc.tensor.matmul(
                            pay_ps[:, 8 + 8 * g + j : 9 + 8 * g + j],
                            g_sb[:, blk], ones[:], start=True, stop=True)
                # Act does the U paycopy (GPSIMD cannot access PSUM; DVE
                # is busy with the diag dot here)
                nc.scalar.activation(
                    pay_sb[:, 8 + 8 * g : 16 + 8 * g],
                    pay_ps[:, 8 + 8 * g : 16 + 8 * g], Copy)
                if not v_done:
                    emit_v()
                    v_done = True

            nc.sync.dma_start(out=outp_d[:], in_=pay_sb[:])

    nc.compile()
    return nc


def _prep_fast(x, lab, w):
    """Class-balanced sharding. Returns (in_maps, nt) or (None, 0) if a
    chunk would need more than 128 distinct classes."""
    import ml_dtypes

    counts = np.bincount(lab, minlength=L)
    order = np.argsort(-counts, kind="stable")
    bins = [[] for _ in range(CORES)]
    for i, cl in enumerate(order):
        r = i % (2 * CORES)
        b = r if r < CORES else 2 * CORES - 1 - r
        bins[b].append(cl)
    seq = [cl for b in bins for cl in b]

    core_classes = [[] for _ in range(CORES)]
    c, filled = 0, 0
    for cl in seq:
        n = int(counts[cl])
        while n > 0:
            take = min(n, NLOC - filled)
            core_classes[c].append((int(cl), take))
            filled += take
            n -= take
            if filled == NLOC:
                c += 1
                filled = 0
    if max(len(cc) for cc in core_classes) > 128:
        return None, 0
    nt = NLOC // 128  # 16

    row_order = np.argsort(lab, kind="stable")
    class_start = np.zeros(L + 1, np.int64)
    class_start[1:] = np.cumsum(counts)
    taken = np.zeros(L, np.int64)

    maps = []
    for cc in core_classes:
        rows = np.empty(NLOC, np.int64)
        lrel = np.empty(NLOC, np.float32)
        wsl = np.zeros((128, D), np.float32)
        hist = np.zeros((128, 1), np.float32)
        pos = 0
        for slot, (cl, take) in enumerate(cc):
            s = class_start[cl] + taken[cl]
            rows[pos : pos + take] = row_order[s : s + take]
            lrel[pos : pos + take] = slot
            taken[cl] += take
            wsl[slot] = w[cl]
            hist[slot, 0] = take
            pos += take
        assert pos == NLOC
        xb = x[rows].astype(ml_dtypes.bfloat16)
        x_t = np.ascontiguousarray(
            xb.reshape(nt, 128, D).transpose(1, 0, 2).reshape(128, nt * D)
        )
        lh = np.empty((128, nt + 1), np.float32)
        lh[:, 0:nt] = lrel.reshape(nt, 128).T
        lh[:, nt] = hist[:, 0]
        maps.append({"x": x_t, "wslice": wsl, "lh": lh})
    return maps, nt


def _combine_fast(results, ng=1):
    """Sum per-core payloads (the unshard step) and finish the scalar."""
    U = np.zeros(D, np.float64)
    V = np.zeros(D, np.float64)
    diag = 0.0
    for r in results:
        pay = np.asarray(r["out_p"], np.float64)
        V += pay[:, 0:8].T.reshape(D)
        for g in range(ng):
            U += pay[:, 8 + 8 * g : 16 + 8 * g].T.reshape(D)
        db = 8 + 8 * ng
        diag += pay[:, db : db + 2 * ng].sum()
    out = diag / N - (U @ V) / (float(N) * float(N))
    return np.float32(out)


def kernel(inputs, labels, W, b):
    global LAST_RESULT
    import os

    # The run path needs the axon trn2 PJRT backend; drop a cpu pin if jax
    # hasn't been initialized yet (the reference is jax-on-cpu friendly).
    if "jax" not in sys.modules and os.environ.get("JAX_PLATFORMS") == "cpu":
        del os.environ["JAX_PLATFORMS"]

    from concourse.bass_utils import run_bass_kernel_spmd

    x = np.ascontiguousarray(np.asarray(inputs, dtype=np.float32))
    lab = np.asarray(labels).astype(np.int64)
    w = np.ascontiguousarray(np.asarray(W, dtype=np.float32))
    assert x.shape == (N, D) and w.shape == (L, D) and lab.shape == (N,)
    assert lab.min() >= 0 and lab.max() < L

    in_maps, nt = _prep_fast(x, lab, w)
    if in_maps is not None:
        key = f"nc_fast_{nt}_{sorted(_FAST_KW.items())}"
        if key not in _CACHE:
            _CACHE[key] = _build_nc_fast(nt=nt, **_FAST_KW)
        nc = _CACHE[key]
        res = run_bass_kernel_spmd(nc, in_maps, list(range(CORES)), **_RUN_KW)
        LAST_RESULT = res
        out = _combine_fast([res.results[c] for c in range(CORES)])
        return np.asarray(out, dtype=np.float32)

    # fallback: bucket-sharded kernel (any label distribution)
    in_maps, btiles = _prep_bucket(x, lab, w)
    assert in_maps is not None, "bucket prep failed"
    key = f"nc_bucket_{btiles}"
    if key not in _CACHE:
        _CACHE[key] = _build_nc_bucket(btiles)
    nc = _CACHE[key]
    res = run_bass_kernel_spmd(nc, in_maps, list(range(CORES)), **_RUN_KW)
    LAST_RESULT = res
    out = np.float32(res.results[0]["out"][0, 0])
    return np.asarray(out, dtype=np.float32)


if __name__ == "__main__":
    import reference

    inp = reference.setup_inputs()
    expected = np.asarray(reference.reference(**inp))
    actual = kernel(**{k: np.asarray(v) for k, v in inp.items()})
    rel = abs(float(actual) - float(expected)) / max(abs(float(expected)), 1e-30)
    print("expected:", expected, "actual:", actual, "rel err:", rel)


# ---------------------------------------------------------------------------
# Bucket-sharded fallback (from the previous kernel): host routes rows to
# cores by class range (125 classes/core). Handles any distribution with
# btiles sized to the fullest bucket.
# ---------------------------------------------------------------------------

LPC = L // CORES            # 125 classes per core
PAY = 17                    # payload cols: U[0:8] | V[8:16] | diag[16]


def _build_nc_bucket(btiles, big_bufs=4):
    import concourse.bacc as bacc
    import concourse.mybir as mybir
    import concourse.tile as tile

    f32 = mybir.dt.float32
    bf16 = mybir.dt.bfloat16
    mult = mybir.AluOpType.mult
    bypass = mybir.AluOpType.bypass
    is_eq = mybir.AluOpType.is_equal

    nc = bacc.Bacc(
        "TRN2",
        target_bir_lowering=False,
        debug=False,
        num_devices=CORES,
    )
    bf16_ = mybir.dt.bfloat16
    x2_d = nc.dram_tensor("x2", [128, btiles * 2 * D], bf16_,
                          kind="ExternalInput")
    w_d = nc.dram_tensor("wslice", [128, D], f32, kind="ExternalInput")
    lrel_d = nc.dram_tensor("lrel", [128, btiles], f32, kind="ExternalInput")
    hist_d = nc.dram_tensor("hist", [128, 1], f32, kind="ExternalInput")
    out_d = nc.dram_tensor("out", [1, 1], f32, kind="ExternalOutput")

    x2_ch = x2_d[:].rearrange("p (c d) -> c p d", c=btiles)

    with tile.TileContext(nc) as tc:
        with (
            tc.tile_pool(name="big", bufs=big_bufs) as big,
            tc.tile_pool(name="small", bufs=1) as small,
            tc.tile_pool(name="eqp", bufs=3) as eqp,
            tc.tile_pool(name="ps", bufs=1, space="PSUM") as ps,
            tc.tile_pool(name="dram", bufs=1, space="DRAM") as dram,
        ):
            slots = small.tile([128, 8 * PAY], f32, tag="slots")
            pay_sb = small.tile([128, PAY], f32, tag="pay_sb")
            nc.vector.memset(pay_sb[:], 0.0)
            ones = small.tile([128, 1], f32, tag="ones")
            nc.vector.memset(ones[:], 1.0)
            junk = small.tile([128, 512], bf16, tag="junk")
            nc.vector.memset(junk[:], 0.0)
            lrel = small.tile([128, btiles], f32, tag="lrel")
            nc.scalar.dma_start(out=lrel[:], in_=lrel_d[:])
            hist = small.tile([128, 1], f32, tag="hist")
            nc.scalar.dma_start(out=hist[:], in_=hist_d[:])
            wsl = small.tile([128, D], f32, tag="wsl")
            nc.gpsimd.dma_start(out=wsl[:], in_=w_d[:])
            iota_i = small.tile([128, 128], mybir.dt.int32, tag="iota_i")
            nc.gpsimd.iota(iota_i[:], pattern=[[1, 128]], channel_multiplier=0)
            iota_f = small.tile([128, 128], f32, tag="iota_f")
            nc.vector.tensor_copy(iota_f[:], iota_i[:])

            p_g0 = ps.tile([128, 512], f32, tag="p_g0")
            p_g1 = ps.tile([128, 512], f32, tag="p_g1")
            pay_ps = ps.tile([128, PAY], f32, tag="pay_ps")
            p_fin = ps.tile([1, 1], f32, tag="p_fin")
            p_warm = ps.tile([1, 512], f32, tag="p_warm")

            for _ in range(7):
                nc.tensor.matmul(p_warm[:], junk[:, 0:1], junk[:],
                                 start=True, stop=True)

            for t in range(btiles):
                first, last = (t == 0), (t == btiles - 1)
                x2 = big.tile([128, 2 * D], bf16, tag="x2", name=f"x2{t}")
                eng = nc.sync if t % 2 == 0 else nc.scalar
                eng.dma_start(out=x2[:], in_=x2_ch[t])
                xh = x2[:, 0:D]
                xl = x2[:, D : 2 * D]
                eq = eqp.tile([128, 128], bf16, tag="eq", name=f"eq{t}")
                nc.vector.tensor_scalar(
                    out=eq[:], in0=iota_f[:], scalar1=lrel[:, t : t + 1],
                    scalar2=None, op0=is_eq,
                )
                nc.tensor.matmul(p_g0[:], eq[:], xh[:, 0:512],
                                 start=first, stop=False)
                nc.tensor.matmul(p_g0[:], eq[:], xl[:, 0:512],
                                 start=False, stop=last)
                nc.tensor.matmul(p_g1[:], eq[:], xh[:, 512:1024],
                                 start=first, stop=False)
                nc.tensor.matmul(p_g1[:], eq[:], xl[:, 512:1024],
                                 start=False, stop=last)

            g_sb = small.tile([128, D], f32, tag="g_sb")
            nc.vector.tensor_copy(g_sb[:, 0:512], p_g0[:])
            nc.vector.tensor_copy(g_sb[:, 512:1024], p_g1[:])

            prod = small.tile([128, D], f32, tag="prod")
            diag_col = small.tile([128, 1], f32, tag="diag_col")
            nc.vector.scalar_tensor_tensor(
                out=prod[:], in0=g_sb[:], scalar=1.0, in1=wsl[:],
                op0=bypass, op1=mult, accum_out=diag_col[:],
            )

            for j in range(8):
                blk = slice(128 * j, 128 * (j + 1))
                nc.tensor.matmul(pay_ps[:, j : j + 1], g_sb[:, blk], ones[:],
                                 start=True, stop=True)
                nc.tensor.matmul(pay_ps[:, 8 + j : 9 + j], wsl[:, blk],
                                 hist[:], start=True, stop=True)
            nc.tensor.matmul(pay_ps[0:1, 16:17], diag_col[:], ones[:],
                             start=True, stop=True)

            nc.vector.tensor_copy(pay_sb[:, 0:16], pay_ps[:, 0:16])
            nc.vector.tensor_copy(pay_sb[0:1, 16:17], pay_ps[0:1, 16:17])
            cc_in = dram.tile([128, PAY], f32, tag="cc_in")
            cc_out = dram.tile([128 * CORES, PAY], f32, tag="cc_out")
            nc.scalar.dma_start(out=cc_in[:], in_=pay_sb[:])
            nc.gpsimd.collective_compute(
                "AllGather",
                bypass,
                replica_groups=[list(range(CORES))],
                ins=[cc_in[:].opt()],
                outs=[cc_out[:].opt()],
            )
            gath = cc_out[:].rearrange("(g p) j -> p g j", g=CORES)
            nc.scalar.dma_start(
                out=slots[:].rearrange("p (g j) -> p g j", g=CORES), in_=gath
            )

            nc.vector.tensor_add(slots[:, 0 : 4 * PAY], slots[:, 0 : 4 * PAY],
                                 slots[:, 4 * PAY : 8 * PAY])
            nc.vector.tensor_add(slots[:, 0 : 2 * PAY], slots[:, 0 : 2 * PAY],
                                 slots[:, 2 * PAY : 4 * PAY])
            nc.vector.tensor_add(slots[:, 0:PAY], slots[:, 0:PAY],
                                 slots[:, PAY : 2 * PAY])

            prod8 = small.tile([128, 8], f32, tag="prod8")
            dot_col = small.tile([128, 1], f32, tag="dot_col")
            nc.vector.scalar_tensor_tensor(
                out=prod8[:], in0=slots[:, 0:8], scalar=1.0,
                in1=slots[:, 8:16], op0=bypass, op1=mult,
                accum_out=dot_col[:],
            )
            nc.tensor.matmul(p_fin[:], dot_col[:], ones[:],
                             start=True, stop=True)
            tmp = small.tile([1, 1], f32, tag="tmp")
            ans = small.tile([1, 1], f32, tag="ans")
            inv_n = 1.0 / float(N)
            nc.vector.tensor_scalar_mul(tmp[:], p_fin[:], inv_n)
            nc.vector.tensor_sub(ans[:], slots[0:1, 16:17], tmp[:])
            nc.vector.tensor_scalar_mul(ans[:], ans[:], inv_n)
            nc.sync.dma_start(out=out_d[:], in_=ans[:])

    nc.compile()
    return nc


def _prep_bucket(x, lab, w):
    """Route rows to cores by label // LPC; btiles sized to the fullest
    bucket (with hi/lo bf16 split so this fallback is near-exact)."""
    core_of = lab // LPC
    counts = np.bincount(core_of, minlength=CORES)
    btiles = max(16, -(-int(counts.max()) // 128))
    maps = []
    for c in range(CORES):
        rows = np.nonzero(core_of == c)[0]
        nb = len(rows)
        import ml_dtypes
        xb = np.zeros((btiles * 128, D), np.float32)
        xb[:nb] = x[rows]
        xh = xb.astype(ml_dtypes.bfloat16)
        xl = (xb - xh.astype(np.float32)).astype(ml_dtypes.bfloat16)
        lrel = np.full((btiles * 128,), -1.0, np.float32)
        lrel[:nb] = (lab[rows] - c * LPC).astype(np.float32)
        wsl = np.zeros((128, D), np.float32)
        wsl[:LPC] = w[c * LPC : (c + 1) * LPC]
        x2 = np.concatenate(
            [xh.reshape(btiles, 128, D), xl.reshape(btiles, 128, D)], axis=2
        ).transpose(1, 0, 2).reshape(128, btiles * 2 * D)

        hist = np.zeros((128, 1), np.float32)
        cnt = np.bincount(lab[rows] - c * LPC, minlength=LPC)
        hist[:LPC, 0] = cnt.astype(np.float32)

        maps.append({
            "x2": np.ascontiguousarray(x2),
            "lrel": np.ascontiguousarray(
                lrel.reshape(btiles, 128).T),
            "wslice": wsl,
            "hist": hist,
        })
    return maps, btiles
